# revision 34
# baseline (speedup 1.0000x reference)
"""Trainium2 Bass kernel for nn_ContrastiveLoss (B=512, ZI=16, T=8, D=128).

Strategy: data-parallel over img batch (64 bi per core), text replicated.

v4 design notes:
  - no device collective: each core emits den_t2i partials [128,32], masked
    E_diag [128,32], and the den_i2t row-sum [1,512]; the host sums partials
    over cores and finishes the (tiny) log-reduce.
  - text arrives host-transposed (d-major, bf16) so the 32 PE transposes and
    f32->bf16 casts disappear; a second row-major bf16 copy feeds the norm
    computation (squares on GpSimd, row-sums on DVE, native Rsqrt on ScalarE).
  - img arrives row-major bf16; normalization is a per-partition scaled copy
    on ScalarE, then 8 PE transposes build im_T.
  - text is NOT normalized before the matmul: 1/|text_row| is constant per
    sim-row (partition) and is folded into the exp scale AP.
  - PSUM evacuation: all-'dve' — one strided reduce_max per q-tile on DVE
    (the only engine that can both read PSUM and reduce; Pool has no max
    ALU op and no PSUM port, so three-engine routing is not possible).
    The small per-tile exp on ScalarE carries accum_out, producing the
    den_t2i column sums for free; E_diag extraction is a GpSimd mask
    multiply + ScalarE Copy-accum, keeping DVE's queue pure MAX.
"""
import os
import numpy as np
import ml_dtypes

B, ZI, T, D = 512, 16, 8, 128
NC = 8
BL = B // NC            # 64 local bi
MLOC = BL * ZI          # 1024 img rows per core
NT = B * T              # 4096 text rows
PT = NT // 128          # 32 text partition-tiles (q)
NG = 4                  # groups of 8 q-tiles
QPG = PT // NG          # 8
DIAG_COEF = -(1.0 + 1.0 / T)

# per-q evacuation route, cycled: see module docstring
_ROUTE_PATTERN = ['dve']


def _route(q):
    return _ROUTE_PATTERN[q % len(_ROUTE_PATTERN)]


_CACHE = {}


def _build_program():
    import concourse.bacc as bacc
    import concourse.mybir as mybir
    import concourse.tile as tile

    f32 = mybir.dt.float32
    bf16 = mybir.dt.bfloat16

    nc = bacc.Bacc("TRN2", num_devices=NC)
    img_rm = nc.declare_dram_parameter("img_rm", [128, 8 * D], bf16,
                                       isOutput=False)
    tn_t = nc.declare_dram_parameter("tn_t", [128, NT], bf16, isOutput=False)
    text_rm = nc.declare_dram_parameter("text_rm", [128, PT * D], bf16,
                                        isOutput=False)
    masks = nc.declare_dram_parameter("masks", [128, PT * BL], bf16,
                                      isOutput=False)
    omc = nc.declare_dram_parameter("omc", [128, PT], f32, isOutput=False)
    ident = nc.declare_dram_parameter("ident", [128, 128], bf16,
                                      isOutput=False)
    out = nc.declare_dram_parameter("out", [128, 2 * PT], f32, isOutput=True)
    out2 = nc.declare_dram_parameter("out2", [1, QPG * BL], f32,
                                     isOutput=True)

    X = mybir.AxisListType.X
    MUL = mybir.AluOpType.mult
    ADD = mybir.AluOpType.add
    MAX = mybir.AluOpType.max
    EXP = mybir.ActivationFunctionType.Exp
    SQRT = mybir.ActivationFunctionType.Sqrt
    SQUARE = mybir.ActivationFunctionType.Square
    COPY = mybir.ActivationFunctionType.Copy

    with tile.TileContext(nc) as tc:
        with (
            tc.tile_pool(name="const", bufs=1) as cp,
            tc.tile_pool(name="sb", bufs=2) as sb,
            tc.tile_pool(name="simp", bufs=6) as sp,
            tc.tile_pool(name="eun", bufs=3) as ep,
            tc.tile_pool(name="ptp", bufs=1, space="PSUM") as ptp,
            tc.tile_pool(name="pmm", bufs=3, space="PSUM") as pmm,
            tc.tile_pool(name="psmall", bufs=1, space="PSUM") as pps,
        ):
            ident_sb = cp.tile([128, 128], bf16)
            nc.sync.dma_start(ident_sb[:], ident[:])
            ones_bf = cp.tile([128, 1], bf16)
            nc.vector.memset(ones_bf[:], 1.0)

            im_rm = cp.tile([128, 8, D], bf16)   # raw img rows, r=k*128+p
            tn_T = cp.tile([128, NT], bf16)      # text d-major [d, row]
            tx_rm = cp.tile([128, PT, D], bf16)  # raw text rows, r=q*128+p
            im_T = cp.tile([128, MLOC], bf16)    # normalized img [d, r]
            invat = cp.tile([128, PT], f32)      # 1/|text_r|, partition=r%128
            den_t = cp.tile([128, PT], f32)      # den_t2i partial cols
            em = cp.tile([128, PT], f32)         # masked E_diag per (q,p)
            em2 = cp.tile([128, PT], f32)        # em + (1 - colmask)

            with tc.high_priority():
                nc.sync.dma_start(im_rm[:], img_rm[:].rearrange(
                    "p (k d) -> p k d", d=D))
            for s in range(8):
                nc.sync.dma_start(
                    tx_rm[:, 4 * s:4 * s + 4, :],
                    text_rm[:, 4 * D * s:4 * D * (s + 1)].rearrange(
                        "p (k d) -> p k d", d=D))
                nc.sync.dma_start(tn_T[:, 512 * s:512 * (s + 1)],
                                  tn_t[:, 512 * s:512 * (s + 1)])
            masks_sb = cp.tile([128, PT * BL], bf16)
            omc_sb = cp.tile([128, PT], f32)

            # ---- img: norms on (V,S), scale on S, transpose on PE ----
            sqi = sb.tile([128, 8, D], bf16, tag="sqi", name="sqi")
            nc.vector.tensor_tensor(sqi[:], im_rm[:], im_rm[:], op=MUL)
            n2i = sb.tile([128, 8], f32, tag="n2i", name="n2i")
            nc.vector.reduce_sum(n2i[:], sqi[:], axis=X)
            rci = sb.tile([128, 8], f32, tag="rci", name="rci")
            nc.vector.reciprocal(rci[:], n2i[:])
            invai = sb.tile([128, 8], f32, tag="invai", name="invai")
            nc.scalar.activation(invai[:], rci[:], SQRT)
            imn = sb.tile([128, 8, D], bf16, tag="imn", name="imn")
            for k in range(8):
                nc.scalar.activation(imn[:, k, :], im_rm[:, k, :], COPY,
                                     scale=invai[:, k:k + 1])
            for h in range(2):
                tp = ptp.tile([128, 4, 128], bf16, tag="tp", name=f"tp{h}")
                for k in range(4):
                    nc.tensor.transpose(tp[:, k, :], imn[:, 4 * h + k, :],
                                        ident_sb[:])
                nc.vector.tensor_copy(
                    im_T[:, 512 * h:512 * (h + 1)],
                    tp[:].rearrange("p k d -> p (k d)"),
                )

            # ---- text: squares on V (early chunks) / G (late chunks),
            # row-sums on V, Sqrt on S ----
            n2t = sb.tile([128, PT], f32, tag="n2t", name="n2t")
            rct = sb.tile([128, PT], f32, tag="rct", name="rct")
            for s in range(8):
                sqt = sb.tile([128, 4, D], bf16, tag="sqt", name=f"sqt{s}")
                eng = nc.vector if s < 4 else nc.gpsimd
                eng.tensor_tensor(sqt[:], tx_rm[:, 4 * s:4 * s + 4, :],
                                  tx_rm[:, 4 * s:4 * s + 4, :], op=MUL)
                nc.vector.reduce_sum(n2t[:, 4 * s:4 * s + 4], sqt[:], axis=X)
                nc.vector.reciprocal(rct[:, 4 * s:4 * s + 4],
                                     n2t[:, 4 * s:4 * s + 4])
                nc.scalar.activation(invat[:, 4 * s:4 * s + 4],
                                     rct[:, 4 * s:4 * s + 4], SQRT)
            # masks arrive after the latency-critical input DMAs
            nc.sync.dma_start(masks_sb[:], masks[:])
            nc.sync.dma_start(omc_sb[:], omc[:])
            # preload the Exp table before the first route exp needs it
            dum = sb.tile([1, 1], f32, tag="dum", name="dum")
            nc.scalar.activation(dum[:], n2i[0:1, 0:1], EXP)

            # ---- main loop ----
            dm_ps = pps.tile([1, QPG * BL], f32, tag="dmx", name="dm_ps")
            for g in range(NG):
                e_g = ep.tile([128, QPG * BL], bf16, tag="eg", name=f"e{g}")
                for qr in range(QPG):
                    q = g * QPG + qr
                    ps = pmm.tile([128, 1024], f32, tag="ps", name=f"ps{q}")
                    for f in range(2):
                        nc.tensor.matmul(
                            ps[:, 512 * f:512 * (f + 1)],
                            lhsT=tn_T[:, 128 * q:128 * (q + 1)],
                            rhs=im_T[:, 512 * f:512 * (f + 1)],
                            start=True, stop=True,
                        )
                    ecols = e_g[:, BL * qr:BL * (qr + 1)]
                    r = _route(q)
                    if r == 'dve':
                        simq = sp.tile([128, BL], f32, tag="simq",
                                       name=f"sim{q}")
                        nc.vector.reduce_max(
                            simq[:],
                            ps[:].rearrange("p (i j) -> p j i", j=BL),
                            axis=X,
                        )
                        nc.scalar.activation(ecols, simq[:], EXP,
                                             scale=invat[:, q:q + 1],
                                             accum_out=den_t[:, q:q + 1])
                    else:
                        eun = ep.tile([128, 1024], bf16, tag="eun",
                                      name=f"eun{q}")
                        nc.scalar.activation(eun[:], ps[:], EXP,
                                             scale=invat[:, q:q + 1])
                        t1 = ep.tile([128, 512], bf16, tag="t1",
                                     name=f"t1_{q}")
                        nc.vector.tensor_tensor(t1[:], eun[:, 0:512],
                                                eun[:, 512:1024], op=MAX)
                        t2 = ep.tile([128, 256], bf16, tag="t2",
                                     name=f"t2_{q}")
                        nc.vector.tensor_tensor(t2[:], t1[:, 0:256],
                                                t1[:, 256:512], op=MAX)
                        t3 = ep.tile([128, 128], bf16, tag="t3",
                                     name=f"t3_{q}")
                        nc.vector.tensor_tensor(t3[:], t2[:, 0:128],
                                                t2[:, 128:256], op=MAX)
                        nc.vector.tensor_tensor(ecols, t3[:, 0:64],
                                                t3[:, 64:128], op=MAX)
                scr2 = sb.tile([128, QPG * BL], bf16, tag="scr2",
                               name=f"scr2_{g}")
                H = QPG * BL // 2
                for hh in range(2):
                    nc.gpsimd.tensor_tensor(
                        scr2[:, H * hh:H * (hh + 1)],
                        e_g[:, H * hh:H * (hh + 1)],
                        masks_sb[:, QPG * BL * g + H * hh:
                                 QPG * BL * g + H * (hh + 1)], op=MUL,
                    )
                    for qr in range(4 * hh, 4 * hh + 4):
                        q = g * QPG + qr
                        emdead = sp.tile([128, BL], bf16, tag="emdead",
                                         name=f"emd{q}")
                        nc.scalar.activation(emdead[:],
                                             scr2[:, BL * qr:BL * (qr + 1)],
                                             COPY, accum_out=em[:, q:q + 1])
                nc.tensor.matmul(
                    dm_ps[:], lhsT=ones_bf[:], rhs=e_g[:],
                    start=(g == 0), stop=(g == NG - 1),
                    skip_group_check=True,
                )
                # stream this group's partials out (em2-add on idle GpSimd)
                gq = slice(QPG * g, QPG * (g + 1))
                nc.gpsimd.tensor_tensor(em2[:, gq], em[:, gq],
                                        omc_sb[:, gq], op=ADD)
                nc.sync.dma_start(out[:, gq], den_t[:, gq])
                nc.sync.dma_start(out[:, PT + QPG * g:PT + QPG * (g + 1)],
                                  em2[:, gq])

            # ---- emit remaining partials ----
            dmv = sb.tile([1, QPG * BL], f32, tag="dmv", name="dmv")
            nc.vector.tensor_copy(dmv[:], dm_ps[:])
            nc.sync.dma_start(out2[:], dmv[:])

    nc.finalize()
    return nc


def _make_mask(c):
    m = np.zeros((128, PT * BL), np.float32)
    p = np.arange(128)
    for k in range(4):
        q = 4 * c + k
        j = 16 * k + p // 8
        m[p, q * BL + j] = 1.0
    return m.astype(ml_dtypes.bfloat16)


def _make_omc(c):
    """1 - colmask: 0 on this core's own 4 q-columns, 1 elsewhere."""
    m = np.ones((128, PT), np.float32)
    m[:, 4 * c:4 * c + 4] = 0.0
    return m


def _get_program():
    if "nc" not in _CACHE:
        _CACHE["nc"] = _build_program()
    return _CACHE["nc"]


def _install_trace_shim():
    """Register the NTFF profile hook that this container's antenv lacks.

    Only used by the local test harness (KERNEL_TRACE=1); the grading
    path never enters here.
    """
    import sys
    import types
    import antenv
    import concourse.bass_utils as bu
    from trn_agent_boot.trn_boot import _ntff_profile_via_ctypes

    if "antenv.axon_hooks" not in sys.modules:
        hook = _ntff_profile_via_ctypes("/opt/axon/libaxon_pjrt.so")
        mod = types.ModuleType("antenv.axon_hooks")
        mod.get_axon_ntff_profile_hook = lambda: hook
        mod.set_axon_ntff_profile_hook = lambda h: None
        sys.modules["antenv.axon_hooks"] = mod
        antenv.axon_hooks = mod
    bu.upload_artifacts = lambda tmpdir: tmpdir


def kernel(img: np.ndarray, text: np.ndarray) -> np.ndarray:
    from concourse.bass_utils import run_bass_kernel_spmd

    nc = _get_program()
    img = np.asarray(img, dtype=np.float32)
    text = np.asarray(text, dtype=np.float32)
    text_flat = text.reshape(NT, D)
    ident = np.eye(128, dtype=ml_dtypes.bfloat16)

    # text: d-major (host transpose) + row-major, both bf16
    tn_t_np = np.ascontiguousarray(text_flat.T).astype(ml_dtypes.bfloat16)
    tx_rm_np = np.ascontiguousarray(
        text_flat.reshape(PT, 128, D).transpose(1, 0, 2)
    ).reshape(128, PT * D).astype(ml_dtypes.bfloat16)

    in_maps = []
    for c in range(NC):
        sh = img[BL * c:BL * (c + 1)].reshape(BL, ZI, D)
        # i-major row order: row r = i*64 + j; partition = r%128, k = r//128
        rows = sh.transpose(1, 0, 2).reshape(MLOC, D)
        img_rm_np = np.ascontiguousarray(
            rows.reshape(8, 128, D).transpose(1, 0, 2)
        ).reshape(128, 8 * D).astype(ml_dtypes.bfloat16)
        in_maps.append({
            "img_rm": img_rm_np,
            "tn_t": tn_t_np,
            "text_rm": tx_rm_np,
            "masks": _make_mask(c),
            "omc": _make_omc(c),
            "ident": ident,
        })

    trace = bool(int(os.environ.get("KERNEL_TRACE", "0")))
    if trace:
        _install_trace_shim()
    r = run_bass_kernel_spmd(nc, in_maps, core_ids=list(range(NC)),
                             trace=trace)
    _CACHE["last_result"] = r
    # unshard: sum den_t2i partials over cores, finish the log-reduce, and
    # add the per-core local contributions
    total = 0.0
    den_t2i = np.zeros((128, PT), np.float64)
    for c in range(NC):
        o = np.asarray(r.results[c]["out"], dtype=np.float64)
        den_t2i += o[:, 0:PT]
        total += DIAG_COEF * float(np.sum(np.log(o[:, PT:2 * PT])))
        dm = np.asarray(r.results[c]["out2"], dtype=np.float64).reshape(
            QPG, BL)
        total += float(np.sum(np.log(dm.sum(axis=0))))
    total += float(np.sum(np.log(den_t2i)))
    return np.asarray(total, dtype=np.float32).reshape(())


# revision 37
# speedup vs baseline: 1.1769x; 1.1769x over previous
"""Trainium2 Bass kernel for nn_ContrastiveLoss (B=512, ZI=16, T=8, D=128).

Strategy: data-parallel over img batch (64 bi per core), text replicated.

v4 design notes:
  - no device collective: each core emits den_t2i partials [128,32], masked
    E_diag [128,32], and the den_i2t row-sum [1,512]; the host sums partials
    over cores and finishes the (tiny) log-reduce.
  - text arrives host-transposed (d-major, bf16) so the 32 PE transposes and
    f32->bf16 casts disappear; a second row-major bf16 copy feeds the norm
    computation (squares on GpSimd, row-sums on DVE, native Rsqrt on ScalarE).
  - img arrives row-major bf16; normalization is a per-partition scaled copy
    on ScalarE, then 8 PE transposes build im_T.
  - text is NOT normalized before the matmul: 1/|text_row| is constant per
    sim-row (partition) and is folded into the exp scale AP.
  - PSUM evacuation: all-'dve' — one strided reduce_max per q-tile on DVE
    (the only engine that can both read PSUM and reduce; Pool has no max
    ALU op and no PSUM port, so three-engine routing is not possible).
    The small per-tile exp on ScalarE carries accum_out, producing the
    den_t2i column sums for free; E_diag extraction is a GpSimd mask
    multiply + ScalarE Copy-accum, keeping DVE's queue pure MAX.
"""
import os
import numpy as np
import ml_dtypes

B, ZI, T, D = 512, 16, 8, 128
NC = 8
BL = B // NC            # 64 local bi
MLOC = BL * ZI          # 1024 img rows per core
NT = B * T              # 4096 text rows
PT = NT // 128          # 32 text partition-tiles (q)
NG = 4                  # groups of 8 q-tiles
QPG = PT // NG          # 8
DIAG_COEF = -(1.0 + 1.0 / T)

# per-q evacuation route, cycled: see module docstring
_ROUTE_PATTERN = ['dve']


def _route(q):
    return _ROUTE_PATTERN[q % len(_ROUTE_PATTERN)]


_CACHE = {}


def _build_program():
    import concourse.bacc as bacc
    import concourse.mybir as mybir
    import concourse.tile as tile

    f32 = mybir.dt.float32
    bf16 = mybir.dt.bfloat16

    nc = bacc.Bacc("TRN2", num_devices=NC)
    img_rm = nc.declare_dram_parameter("img_rm", [128, 8 * D], bf16,
                                       isOutput=False)
    tn_t = nc.declare_dram_parameter("tn_t", [128, NT], bf16, isOutput=False)
    text_rm = nc.declare_dram_parameter("text_rm", [128, PT * D], bf16,
                                        isOutput=False)
    masks = nc.declare_dram_parameter("masks", [128, PT * BL], bf16,
                                      isOutput=False)
    omc = nc.declare_dram_parameter("omc", [128, PT], f32, isOutput=False)
    ident = nc.declare_dram_parameter("ident", [128, 128], bf16,
                                      isOutput=False)
    out = nc.declare_dram_parameter("out", [128, 2 * PT], f32, isOutput=True)
    out2 = nc.declare_dram_parameter("out2", [1, QPG * BL], f32,
                                     isOutput=True)

    X = mybir.AxisListType.X
    MUL = mybir.AluOpType.mult
    ADD = mybir.AluOpType.add
    MAX = mybir.AluOpType.max
    EXP = mybir.ActivationFunctionType.Exp
    SQRT = mybir.ActivationFunctionType.Sqrt
    SQUARE = mybir.ActivationFunctionType.Square
    COPY = mybir.ActivationFunctionType.Copy

    with tile.TileContext(nc) as tc:
        with (
            tc.tile_pool(name="const", bufs=1) as cp,
            tc.tile_pool(name="sb", bufs=2) as sb,
            tc.tile_pool(name="simp", bufs=6) as sp,
            tc.tile_pool(name="eun", bufs=3) as ep,
            tc.tile_pool(name="ptp", bufs=1, space="PSUM") as ptp,
            tc.tile_pool(name="pmm", bufs=3, space="PSUM") as pmm,
            tc.tile_pool(name="psmall", bufs=1, space="PSUM") as pps,
        ):
            ident_sb = cp.tile([128, 128], bf16)
            nc.sync.dma_start(ident_sb[:], ident[:])
            ones_bf = cp.tile([128, 1], bf16)
            nc.vector.memset(ones_bf[:], 1.0)

            im_rm = cp.tile([128, 8, D], bf16)   # raw img rows, r=k*128+p
            tn_T = cp.tile([128, NT], bf16)      # text d-major [d, row]
            tx_rm = cp.tile([128, PT, D], bf16)  # raw text rows, r=q*128+p
            im_T = cp.tile([128, MLOC], bf16)    # normalized img [d, r]
            invat = cp.tile([128, PT], f32)      # 1/|text_r|, partition=r%128
            den_t = cp.tile([128, PT], f32)      # den_t2i partial cols
            em = cp.tile([128, PT], f32)         # masked E_diag per (q,p)
            em2 = cp.tile([128, PT], f32)        # em + (1 - colmask)

            with tc.high_priority():
                nc.sync.dma_start(im_rm[:], img_rm[:].rearrange(
                    "p (k d) -> p k d", d=D))
            for s in range(8):
                nc.sync.dma_start(
                    tx_rm[:, 4 * s:4 * s + 4, :],
                    text_rm[:, 4 * D * s:4 * D * (s + 1)].rearrange(
                        "p (k d) -> p k d", d=D))
                nc.sync.dma_start(tn_T[:, 512 * s:512 * (s + 1)],
                                  tn_t[:, 512 * s:512 * (s + 1)])
            masks_sb = cp.tile([128, PT * BL], bf16)
            nc.sync.dma_start(masks_sb[:], masks[:])
            omc_sb = cp.tile([128, PT], f32)
            nc.sync.dma_start(omc_sb[:], omc[:])

            # ---- img: norms on (V,S), scale on S, transpose on PE ----
            sqi = sb.tile([128, 8, D], bf16, tag="sqi", name="sqi")
            nc.vector.tensor_tensor(sqi[:], im_rm[:], im_rm[:], op=MUL)
            n2i = sb.tile([128, 8], f32, tag="n2i", name="n2i")
            nc.vector.reduce_sum(n2i[:], sqi[:], axis=X)
            rci = sb.tile([128, 8], f32, tag="rci", name="rci")
            nc.vector.reciprocal(rci[:], n2i[:])
            invai = sb.tile([128, 8], f32, tag="invai", name="invai")
            nc.scalar.activation(invai[:], rci[:], SQRT)
            imn = sb.tile([128, 8, D], bf16, tag="imn", name="imn")
            for k in range(8):
                nc.scalar.activation(imn[:, k, :], im_rm[:, k, :], COPY,
                                     scale=invai[:, k:k + 1])
            for h in range(2):
                tp = ptp.tile([128, 4, 128], bf16, tag="tp", name=f"tp{h}")
                for k in range(4):
                    nc.tensor.transpose(tp[:, k, :], imn[:, 4 * h + k, :],
                                        ident_sb[:])
                nc.vector.tensor_copy(
                    im_T[:, 512 * h:512 * (h + 1)],
                    tp[:].rearrange("p k d -> p (k d)"),
                )

            # ---- text: squares on V (early chunks) / G (late chunks),
            # row-sums on V, Sqrt on S ----
            n2t = sb.tile([128, PT], f32, tag="n2t", name="n2t")
            rct = sb.tile([128, PT], f32, tag="rct", name="rct")
            for s in range(8):
                sqt = sb.tile([128, 4, D], bf16, tag="sqt", name=f"sqt{s}")
                nc.vector.tensor_tensor(sqt[:], tx_rm[:, 4 * s:4 * s + 4, :],
                                        tx_rm[:, 4 * s:4 * s + 4, :], op=MUL)
                nc.vector.reduce_sum(n2t[:, 4 * s:4 * s + 4], sqt[:], axis=X)
                nc.vector.reciprocal(rct[:, 4 * s:4 * s + 4],
                                     n2t[:, 4 * s:4 * s + 4])
                nc.scalar.activation(invat[:, 4 * s:4 * s + 4],
                                     rct[:, 4 * s:4 * s + 4], SQRT)
            # preload the Exp table before the first route exp needs it
            dum = sb.tile([1, 1], f32, tag="dum", name="dum")
            nc.scalar.activation(dum[:], n2i[0:1, 0:1], EXP)

            # ---- main loop ----
            dm_ps = pps.tile([1, QPG * BL], f32, tag="dmx", name="dm_ps")
            for g in range(NG):
                e_g = ep.tile([128, QPG * BL], bf16, tag="eg", name=f"e{g}")
                for qr in range(QPG):
                    q = g * QPG + qr
                    ps = pmm.tile([128, 1024], f32, tag="ps", name=f"ps{q}")
                    for f in range(2):
                        nc.tensor.matmul(
                            ps[:, 512 * f:512 * (f + 1)],
                            lhsT=tn_T[:, 128 * q:128 * (q + 1)],
                            rhs=im_T[:, 512 * f:512 * (f + 1)],
                            start=True, stop=True,
                        )
                    ecols = e_g[:, BL * qr:BL * (qr + 1)]
                    r = _route(q)
                    if r == 'dve':
                        simq = sp.tile([128, BL], f32, tag="simq",
                                       name=f"sim{q}")
                        nc.vector.reduce_max(
                            simq[:],
                            ps[:].rearrange("p (i j) -> p j i", j=BL),
                            axis=X,
                        )
                        nc.scalar.activation(ecols, simq[:], EXP,
                                             scale=invat[:, q:q + 1],
                                             accum_out=den_t[:, q:q + 1])
                    else:
                        eun = ep.tile([128, 1024], bf16, tag="eun",
                                      name=f"eun{q}")
                        nc.scalar.activation(eun[:], ps[:], EXP,
                                             scale=invat[:, q:q + 1])
                        t1 = ep.tile([128, 512], bf16, tag="t1",
                                     name=f"t1_{q}")
                        nc.vector.tensor_tensor(t1[:], eun[:, 0:512],
                                                eun[:, 512:1024], op=MAX)
                        t2 = ep.tile([128, 256], bf16, tag="t2",
                                     name=f"t2_{q}")
                        nc.vector.tensor_tensor(t2[:], t1[:, 0:256],
                                                t1[:, 256:512], op=MAX)
                        t3 = ep.tile([128, 128], bf16, tag="t3",
                                     name=f"t3_{q}")
                        nc.vector.tensor_tensor(t3[:], t2[:, 0:128],
                                                t2[:, 128:256], op=MAX)
                        nc.vector.tensor_tensor(ecols, t3[:, 0:64],
                                                t3[:, 64:128], op=MAX)
                scr2 = sb.tile([128, QPG * BL], bf16, tag="scr2",
                               name=f"scr2_{g}")
                H = QPG * BL // 2
                for hh in range(2):
                    nc.gpsimd.tensor_tensor(
                        scr2[:, H * hh:H * (hh + 1)],
                        e_g[:, H * hh:H * (hh + 1)],
                        masks_sb[:, QPG * BL * g + H * hh:
                                 QPG * BL * g + H * (hh + 1)], op=MUL,
                    )
                    for qr in range(4 * hh, 4 * hh + 4):
                        q = g * QPG + qr
                        emdead = sp.tile([128, BL], bf16, tag="emdead",
                                         name=f"emd{q}")
                        nc.scalar.activation(emdead[:],
                                             scr2[:, BL * qr:BL * (qr + 1)],
                                             COPY, accum_out=em[:, q:q + 1])
                nc.tensor.matmul(
                    dm_ps[:], lhsT=ones_bf[:], rhs=e_g[:],
                    start=(g == 0), stop=(g == NG - 1),
                    skip_group_check=True,
                )

            # ---- emit partials ----
            nc.vector.tensor_tensor(em2[:], em[:], omc_sb[:], op=ADD)
            dmv = sb.tile([1, QPG * BL], f32, tag="dmv", name="dmv")
            nc.vector.tensor_copy(dmv[:], dm_ps[:])
            nc.sync.dma_start(out[:, 0:PT], den_t[:])
            nc.sync.dma_start(out[:, PT:2 * PT], em2[:])
            nc.sync.dma_start(out2[:], dmv[:])

    nc.finalize()
    return nc


def _make_mask(c):
    m = np.zeros((128, PT * BL), np.float32)
    p = np.arange(128)
    for k in range(4):
        q = 4 * c + k
        j = 16 * k + p // 8
        m[p, q * BL + j] = 1.0
    return m.astype(ml_dtypes.bfloat16)


def _make_omc(c):
    """1 - colmask: 0 on this core's own 4 q-columns, 1 elsewhere."""
    m = np.ones((128, PT), np.float32)
    m[:, 4 * c:4 * c + 4] = 0.0
    return m


def _get_program():
    if "nc" not in _CACHE:
        _CACHE["nc"] = _build_program()
    return _CACHE["nc"]


def _install_trace_shim():
    """Register the NTFF profile hook that this container's antenv lacks.

    Only used by the local test harness (KERNEL_TRACE=1); the grading
    path never enters here.
    """
    import sys
    import types
    import antenv
    import concourse.bass_utils as bu
    from trn_agent_boot.trn_boot import _ntff_profile_via_ctypes

    if "antenv.axon_hooks" not in sys.modules:
        hook = _ntff_profile_via_ctypes("/opt/axon/libaxon_pjrt.so")
        mod = types.ModuleType("antenv.axon_hooks")
        mod.get_axon_ntff_profile_hook = lambda: hook
        mod.set_axon_ntff_profile_hook = lambda h: None
        sys.modules["antenv.axon_hooks"] = mod
        antenv.axon_hooks = mod
    bu.upload_artifacts = lambda tmpdir: tmpdir


def kernel(img: np.ndarray, text: np.ndarray) -> np.ndarray:
    from concourse.bass_utils import run_bass_kernel_spmd

    nc = _get_program()
    img = np.asarray(img, dtype=np.float32)
    text = np.asarray(text, dtype=np.float32)
    text_flat = text.reshape(NT, D)
    ident = np.eye(128, dtype=ml_dtypes.bfloat16)

    # text: d-major (host transpose) + row-major, both bf16
    tn_t_np = np.ascontiguousarray(text_flat.T).astype(ml_dtypes.bfloat16)
    tx_rm_np = np.ascontiguousarray(
        text_flat.reshape(PT, 128, D).transpose(1, 0, 2)
    ).reshape(128, PT * D).astype(ml_dtypes.bfloat16)

    in_maps = []
    for c in range(NC):
        sh = img[BL * c:BL * (c + 1)].reshape(BL, ZI, D)
        # i-major row order: row r = i*64 + j; partition = r%128, k = r//128
        rows = sh.transpose(1, 0, 2).reshape(MLOC, D)
        img_rm_np = np.ascontiguousarray(
            rows.reshape(8, 128, D).transpose(1, 0, 2)
        ).reshape(128, 8 * D).astype(ml_dtypes.bfloat16)
        in_maps.append({
            "img_rm": img_rm_np,
            "tn_t": tn_t_np,
            "text_rm": tx_rm_np,
            "masks": _make_mask(c),
            "omc": _make_omc(c),
            "ident": ident,
        })

    trace = bool(int(os.environ.get("KERNEL_TRACE", "0")))
    if trace:
        _install_trace_shim()
    r = run_bass_kernel_spmd(nc, in_maps, core_ids=list(range(NC)),
                             trace=trace)
    _CACHE["last_result"] = r
    # unshard: sum den_t2i partials over cores, finish the log-reduce, and
    # add the per-core local contributions
    total = 0.0
    den_t2i = np.zeros((128, PT), np.float64)
    for c in range(NC):
        o = np.asarray(r.results[c]["out"], dtype=np.float64)
        den_t2i += o[:, 0:PT]
        total += DIAG_COEF * float(np.sum(np.log(o[:, PT:2 * PT])))
        dm = np.asarray(r.results[c]["out2"], dtype=np.float64).reshape(
            QPG, BL)
        total += float(np.sum(np.log(dm.sum(axis=0))))
    total += float(np.sum(np.log(den_t2i)))
    return np.asarray(total, dtype=np.float32).reshape(())


# revision 38
# speedup vs baseline: 1.1806x; 1.0032x over previous
"""Trainium2 Bass kernel for nn_ContrastiveLoss (B=512, ZI=16, T=8, D=128).

Strategy: data-parallel over img batch (64 bi per core), text replicated.

v4 design notes:
  - no device collective: each core emits den_t2i partials [128,32], masked
    E_diag [128,32], and the den_i2t row-sum [1,512]; the host sums partials
    over cores and finishes the (tiny) log-reduce.
  - text arrives host-transposed (d-major, bf16) so the 32 PE transposes and
    f32->bf16 casts disappear; a second row-major bf16 copy feeds the norm
    computation (squares on GpSimd, row-sums on DVE, native Rsqrt on ScalarE).
  - img arrives row-major bf16; normalization is a per-partition scaled copy
    on ScalarE, then 8 PE transposes build im_T.
  - text is NOT normalized before the matmul: 1/|text_row| is constant per
    sim-row (partition) and is folded into the exp scale AP.
  - PSUM evacuation: all-'dve' — one strided reduce_max per q-tile on DVE
    (the only engine that can both read PSUM and reduce; Pool has no max
    ALU op and no PSUM port, so three-engine routing is not possible).
    The small per-tile exp on ScalarE carries accum_out, producing the
    den_t2i column sums for free; E_diag extraction is a GpSimd mask
    multiply + ScalarE Copy-accum, keeping DVE's queue pure MAX.
"""
import os
import numpy as np
import ml_dtypes

B, ZI, T, D = 512, 16, 8, 128
NC = 8
BL = B // NC            # 64 local bi
MLOC = BL * ZI          # 1024 img rows per core
NT = B * T              # 4096 text rows
PT = NT // 128          # 32 text partition-tiles (q)
NG = 4                  # groups of 8 q-tiles
QPG = PT // NG          # 8
DIAG_COEF = -(1.0 + 1.0 / T)

# per-q evacuation route, cycled: see module docstring
_ROUTE_PATTERN = ['dve']


def _route(q):
    return _ROUTE_PATTERN[q % len(_ROUTE_PATTERN)]


_CACHE = {}


def _build_program():
    import concourse.bacc as bacc
    import concourse.mybir as mybir
    import concourse.tile as tile

    f32 = mybir.dt.float32
    bf16 = mybir.dt.bfloat16

    nc = bacc.Bacc("TRN2", num_devices=NC)
    img_rm = nc.declare_dram_parameter("img_rm", [128, 8 * D], bf16,
                                       isOutput=False)
    tn_t = nc.declare_dram_parameter("tn_t", [128, NT], bf16, isOutput=False)
    text_rm = nc.declare_dram_parameter("text_rm", [128, PT * D], bf16,
                                        isOutput=False)
    masks = nc.declare_dram_parameter("masks", [128, PT * BL], bf16,
                                      isOutput=False)
    omc = nc.declare_dram_parameter("omc", [128, PT], f32, isOutput=False)
    ident = nc.declare_dram_parameter("ident", [128, 128], bf16,
                                      isOutput=False)
    out = nc.declare_dram_parameter("out", [128, 2 * PT], f32, isOutput=True)
    out2 = nc.declare_dram_parameter("out2", [1, QPG * BL], f32,
                                     isOutput=True)

    X = mybir.AxisListType.X
    MUL = mybir.AluOpType.mult
    ADD = mybir.AluOpType.add
    MAX = mybir.AluOpType.max
    EXP = mybir.ActivationFunctionType.Exp
    SQRT = mybir.ActivationFunctionType.Sqrt
    SQUARE = mybir.ActivationFunctionType.Square
    COPY = mybir.ActivationFunctionType.Copy

    with tile.TileContext(nc) as tc:
        with (
            tc.tile_pool(name="const", bufs=1) as cp,
            tc.tile_pool(name="sb", bufs=2) as sb,
            tc.tile_pool(name="simp", bufs=6) as sp,
            tc.tile_pool(name="eun", bufs=3) as ep,
            tc.tile_pool(name="ptp", bufs=1, space="PSUM") as ptp,
            tc.tile_pool(name="pmm", bufs=3, space="PSUM") as pmm,
            tc.tile_pool(name="psmall", bufs=1, space="PSUM") as pps,
        ):
            ident_sb = cp.tile([128, 128], bf16)
            nc.sync.dma_start(ident_sb[:], ident[:])
            ones_bf = cp.tile([128, 1], bf16)
            nc.vector.memset(ones_bf[:], 1.0)

            im_rm = cp.tile([128, 8, D], bf16)   # raw img rows, r=k*128+p
            tn_T = cp.tile([128, NT], bf16)      # text d-major [d, row]
            tx_rm = cp.tile([128, PT, D], bf16)  # raw text rows, r=q*128+p
            im_T = cp.tile([128, MLOC], bf16)    # normalized img [d, r]
            invat = cp.tile([128, PT], f32)      # 1/|text_r|, partition=r%128
            den_t = cp.tile([128, PT], f32)      # den_t2i partial cols
            em = cp.tile([128, PT], f32)         # masked E_diag per (q,p)
            em2 = cp.tile([128, PT], f32)        # em + (1 - colmask)

            with tc.high_priority():
                nc.sync.dma_start(im_rm[:], img_rm[:].rearrange(
                    "p (k d) -> p k d", d=D))
            for s in range(8):
                nc.sync.dma_start(
                    tx_rm[:, 4 * s:4 * s + 4, :],
                    text_rm[:, 4 * D * s:4 * D * (s + 1)].rearrange(
                        "p (k d) -> p k d", d=D))
                nc.sync.dma_start(tn_T[:, 512 * s:512 * (s + 1)],
                                  tn_t[:, 512 * s:512 * (s + 1)])
            masks_sb = cp.tile([128, PT * BL], bf16)
            nc.sync.dma_start(masks_sb[:], masks[:])
            omc_sb = cp.tile([128, PT], f32)
            nc.sync.dma_start(omc_sb[:], omc[:])

            # ---- img: norms on (V,S), scale on S, transpose on PE ----
            sqi = sb.tile([128, 8, D], bf16, tag="sqi", name="sqi")
            nc.vector.tensor_tensor(sqi[:], im_rm[:], im_rm[:], op=MUL)
            n2i = sb.tile([128, 8], f32, tag="n2i", name="n2i")
            nc.vector.reduce_sum(n2i[:], sqi[:], axis=X)
            rci = sb.tile([128, 8], f32, tag="rci", name="rci")
            nc.vector.reciprocal(rci[:], n2i[:])
            invai = sb.tile([128, 8], f32, tag="invai", name="invai")
            nc.scalar.activation(invai[:], rci[:], SQRT)
            imn = sb.tile([128, 8, D], bf16, tag="imn", name="imn")
            for k in range(8):
                nc.scalar.activation(imn[:, k, :], im_rm[:, k, :], COPY,
                                     scale=invai[:, k:k + 1])
            for h in range(2):
                tp = ptp.tile([128, 4, 128], bf16, tag="tp", name=f"tp{h}")
                for k in range(4):
                    nc.tensor.transpose(tp[:, k, :], imn[:, 4 * h + k, :],
                                        ident_sb[:])
                nc.scalar.activation(
                    im_T[:, 512 * h:512 * (h + 1)],
                    tp[:].rearrange("p k d -> p (k d)"), COPY)

            # ---- text: squares on V (early chunks) / G (late chunks),
            # row-sums on V, Sqrt on S ----
            n2t = sb.tile([128, PT], f32, tag="n2t", name="n2t")
            rct = sb.tile([128, PT], f32, tag="rct", name="rct")
            for s in range(8):
                sqt = sb.tile([128, 4, D], bf16, tag="sqt", name=f"sqt{s}")
                nc.vector.tensor_tensor(sqt[:], tx_rm[:, 4 * s:4 * s + 4, :],
                                        tx_rm[:, 4 * s:4 * s + 4, :], op=MUL)
                nc.vector.reduce_sum(n2t[:, 4 * s:4 * s + 4], sqt[:], axis=X)
                nc.vector.reciprocal(rct[:, 4 * s:4 * s + 4],
                                     n2t[:, 4 * s:4 * s + 4])
                nc.scalar.activation(invat[:, 4 * s:4 * s + 4],
                                     rct[:, 4 * s:4 * s + 4], SQRT)
            # preload the Exp table before the first route exp needs it
            dum = sb.tile([1, 1], f32, tag="dum", name="dum")
            nc.scalar.activation(dum[:], n2i[0:1, 0:1], EXP)

            # ---- main loop ----
            dm_ps = pps.tile([1, QPG * BL], f32, tag="dmx", name="dm_ps")
            for g in range(NG):
                e_g = ep.tile([128, QPG * BL], bf16, tag="eg", name=f"e{g}")
                for qr in range(QPG):
                    q = g * QPG + qr
                    ps = pmm.tile([128, 1024], f32, tag="ps", name=f"ps{q}")
                    for f in range(2):
                        nc.tensor.matmul(
                            ps[:, 512 * f:512 * (f + 1)],
                            lhsT=tn_T[:, 128 * q:128 * (q + 1)],
                            rhs=im_T[:, 512 * f:512 * (f + 1)],
                            start=True, stop=True,
                        )
                    ecols = e_g[:, BL * qr:BL * (qr + 1)]
                    r = _route(q)
                    if r == 'dve':
                        simq = sp.tile([128, BL], f32, tag="simq",
                                       name=f"sim{q}")
                        nc.vector.reduce_max(
                            simq[:],
                            ps[:].rearrange("p (i j) -> p j i", j=BL),
                            axis=X,
                        )
                        nc.scalar.activation(ecols, simq[:], EXP,
                                             scale=invat[:, q:q + 1],
                                             accum_out=den_t[:, q:q + 1])
                    else:
                        eun = ep.tile([128, 1024], bf16, tag="eun",
                                      name=f"eun{q}")
                        nc.scalar.activation(eun[:], ps[:], EXP,
                                             scale=invat[:, q:q + 1])
                        t1 = ep.tile([128, 512], bf16, tag="t1",
                                     name=f"t1_{q}")
                        nc.vector.tensor_tensor(t1[:], eun[:, 0:512],
                                                eun[:, 512:1024], op=MAX)
                        t2 = ep.tile([128, 256], bf16, tag="t2",
                                     name=f"t2_{q}")
                        nc.vector.tensor_tensor(t2[:], t1[:, 0:256],
                                                t1[:, 256:512], op=MAX)
                        t3 = ep.tile([128, 128], bf16, tag="t3",
                                     name=f"t3_{q}")
                        nc.vector.tensor_tensor(t3[:], t2[:, 0:128],
                                                t2[:, 128:256], op=MAX)
                        nc.vector.tensor_tensor(ecols, t3[:, 0:64],
                                                t3[:, 64:128], op=MAX)
                scr2 = sb.tile([128, QPG * BL], bf16, tag="scr2",
                               name=f"scr2_{g}")
                H = QPG * BL // 2
                for hh in range(2):
                    nc.gpsimd.tensor_tensor(
                        scr2[:, H * hh:H * (hh + 1)],
                        e_g[:, H * hh:H * (hh + 1)],
                        masks_sb[:, QPG * BL * g + H * hh:
                                 QPG * BL * g + H * (hh + 1)], op=MUL,
                    )
                    for qr in range(4 * hh, 4 * hh + 4):
                        q = g * QPG + qr
                        emdead = sp.tile([128, BL], bf16, tag="emdead",
                                         name=f"emd{q}")
                        nc.scalar.activation(emdead[:],
                                             scr2[:, BL * qr:BL * (qr + 1)],
                                             COPY, accum_out=em[:, q:q + 1])
                nc.tensor.matmul(
                    dm_ps[:], lhsT=ones_bf[:], rhs=e_g[:],
                    start=(g == 0), stop=(g == NG - 1),
                    skip_group_check=True,
                )

            # ---- emit partials ----
            nc.vector.tensor_tensor(em2[:], em[:], omc_sb[:], op=ADD)
            dmv = sb.tile([1, QPG * BL], f32, tag="dmv", name="dmv")
            nc.vector.tensor_copy(dmv[:], dm_ps[:])
            nc.sync.dma_start(out[:, 0:PT], den_t[:])
            nc.sync.dma_start(out[:, PT:2 * PT], em2[:])
            nc.sync.dma_start(out2[:], dmv[:])

    nc.finalize()
    return nc


def _make_mask(c):
    m = np.zeros((128, PT * BL), np.float32)
    p = np.arange(128)
    for k in range(4):
        q = 4 * c + k
        j = 16 * k + p // 8
        m[p, q * BL + j] = 1.0
    return m.astype(ml_dtypes.bfloat16)


def _make_omc(c):
    """1 - colmask: 0 on this core's own 4 q-columns, 1 elsewhere."""
    m = np.ones((128, PT), np.float32)
    m[:, 4 * c:4 * c + 4] = 0.0
    return m


def _get_program():
    if "nc" not in _CACHE:
        _CACHE["nc"] = _build_program()
    return _CACHE["nc"]


def _install_trace_shim():
    """Register the NTFF profile hook that this container's antenv lacks.

    Only used by the local test harness (KERNEL_TRACE=1); the grading
    path never enters here.
    """
    import sys
    import types
    import antenv
    import concourse.bass_utils as bu
    from trn_agent_boot.trn_boot import _ntff_profile_via_ctypes

    if "antenv.axon_hooks" not in sys.modules:
        hook = _ntff_profile_via_ctypes("/opt/axon/libaxon_pjrt.so")
        mod = types.ModuleType("antenv.axon_hooks")
        mod.get_axon_ntff_profile_hook = lambda: hook
        mod.set_axon_ntff_profile_hook = lambda h: None
        sys.modules["antenv.axon_hooks"] = mod
        antenv.axon_hooks = mod
    bu.upload_artifacts = lambda tmpdir: tmpdir


def kernel(img: np.ndarray, text: np.ndarray) -> np.ndarray:
    from concourse.bass_utils import run_bass_kernel_spmd

    nc = _get_program()
    img = np.asarray(img, dtype=np.float32)
    text = np.asarray(text, dtype=np.float32)
    text_flat = text.reshape(NT, D)
    ident = np.eye(128, dtype=ml_dtypes.bfloat16)

    # text: d-major (host transpose) + row-major, both bf16
    tn_t_np = np.ascontiguousarray(text_flat.T).astype(ml_dtypes.bfloat16)
    tx_rm_np = np.ascontiguousarray(
        text_flat.reshape(PT, 128, D).transpose(1, 0, 2)
    ).reshape(128, PT * D).astype(ml_dtypes.bfloat16)

    in_maps = []
    for c in range(NC):
        sh = img[BL * c:BL * (c + 1)].reshape(BL, ZI, D)
        # i-major row order: row r = i*64 + j; partition = r%128, k = r//128
        rows = sh.transpose(1, 0, 2).reshape(MLOC, D)
        img_rm_np = np.ascontiguousarray(
            rows.reshape(8, 128, D).transpose(1, 0, 2)
        ).reshape(128, 8 * D).astype(ml_dtypes.bfloat16)
        in_maps.append({
            "img_rm": img_rm_np,
            "tn_t": tn_t_np,
            "text_rm": tx_rm_np,
            "masks": _make_mask(c),
            "omc": _make_omc(c),
            "ident": ident,
        })

    trace = bool(int(os.environ.get("KERNEL_TRACE", "0")))
    if trace:
        _install_trace_shim()
    r = run_bass_kernel_spmd(nc, in_maps, core_ids=list(range(NC)),
                             trace=trace)
    _CACHE["last_result"] = r
    # unshard: sum den_t2i partials over cores, finish the log-reduce, and
    # add the per-core local contributions
    total = 0.0
    den_t2i = np.zeros((128, PT), np.float64)
    for c in range(NC):
        o = np.asarray(r.results[c]["out"], dtype=np.float64)
        den_t2i += o[:, 0:PT]
        total += DIAG_COEF * float(np.sum(np.log(o[:, PT:2 * PT])))
        dm = np.asarray(r.results[c]["out2"], dtype=np.float64).reshape(
            QPG, BL)
        total += float(np.sum(np.log(dm.sum(axis=0))))
    total += float(np.sum(np.log(den_t2i)))
    return np.asarray(total, dtype=np.float32).reshape(())
